# revision 1
# baseline (speedup 1.0000x reference)
"""Trainium2 Bass kernel for depthwise-multiplier conv + ReLU + per-out-channel
1x1 combine (nn_Comb_70016556859799).

Math (reference):
  out[b,o,p,q] = bc[o] + sum_i Wc[o,i] * relu( sum_{dy,dx} Wf[o,i,dy,dx]*x[b,i,p+dy,q+dx] + bf[o,i] )

Sharding: 8 cores = (batch b in 0..3) x (H half in 0..1). Each core computes
out[b, :, 63*h : 63*h+63, :] from x[b, :, 63*h : 63*h+65, :].

Per-core dataflow:
  - conv: z_i[o, pix] = Wf[:,i]^T(9x64) @ Xs_i(9 x pix) where Xs_i is a
    host-precomputed im2col layout (one clean strided DMA per channel). Four
    input channels run concurrently on the PE via 32-row/64-col tile packing.
  - relu+bias: ScalarE activation / VectorE tensor_scalar, PSUM->SBUF, bf16.
  - combine: out[o,pix] += Wc[o,i]*r_i[o,pix] as matmuls with stacked-diagonal
    lhsT (two channels per matmul, K=128) accumulated in PSUM; two chunks run
    concurrently via column-group packing.

Pixels are processed in 4-row x 128-col chunks (=512 f32 = one PSUM bank);
columns 126,127 are zero padding. Rows 0..62 are the real output rows; the
im2col buffer is padded to 64 rows so every chunk is uniform.
"""
import numpy as np
import ml_dtypes

import concourse.bass as bass
import concourse.mybir as mybir
from concourse import tile
from concourse.bass_utils import run_bass_kernel_spmd

BF16 = mybir.dt.bfloat16
F32 = mybir.dt.float32
npbf16 = ml_dtypes.bfloat16

B, FIN, FOUT, KK, H, W = 4, 64, 64, 3, 128, 128
HO, WO = H - KK + 1, W - KK + 1          # 126, 126
RPC = HO // 2                             # 63 output rows per core
HALO = RPC + KK - 1                       # 65 input rows per core
NQ = FIN // 4                             # 16 quads of input channels
XR, XC = 64, 128                          # padded im2col rows/cols per core
CGRPS = [0, 16, 32, 48]                   # 4 chunk groups x 16 rows


def _hoist_extra_waits(nc):
    """Walrus supports only one sync-wait command per instruction. Tile can
    emit several (multiple producer procs). Hoist all but the last wait onto
    fresh same-engine NoOp instructions placed immediately before -- the waits
    still execute on the same engine sequencer in the same order, so the
    synchronization semantics are unchanged."""
    import copy
    n_hoist = 0
    for blk in nc.m.functions[0].blocks:
        newinsts = []
        for inst in blk.instructions:
            si = getattr(inst, "sync_info", None)
            ow = list(si.on_wait) if si is not None and si.on_wait else []
            if len(ow) > 1:
                for wi, w in enumerate(ow[:-1]):
                    nop = mybir.InstNoOp(
                        name=f"{inst.name}_hw{wi}",
                        text_hint="hoisted_wait",
                        bass_nofuse=True,
                    )
                    nop.engine = inst.engine
                    nsi = copy.deepcopy(si)
                    nsi.on_wait = [w]
                    if getattr(nsi, "on_update", None):
                        nsi.on_update = []
                    nop.sync_info = nsi
                    newinsts.append(nop)
                    n_hoist += 1
                si.on_wait = [ow[-1]]
            newinsts.append(inst)
        blk.instructions = newinsts
    return n_hoist


def _build(hoist=True):
    nc = bass.Bass()
    xs_d = nc.declare_dram_parameter("xs", [FIN, 9, XR, XC], BF16, isOutput=False)
    wq_d = nc.declare_dram_parameter("wq", [NQ, 128, FOUT], BF16, isOutput=False)
    ds_d = nc.declare_dram_parameter("ds", [2 * NQ, 128, FOUT], BF16, isOutput=False)
    bfp_d = nc.declare_dram_parameter("bfp", [2 * NQ, 128, 1], F32, isOutput=False)
    bc2_d = nc.declare_dram_parameter("bc2", [128, 1], F32, isOutput=False)
    out_d = nc.declare_dram_parameter("out", [FOUT, RPC, XC], F32, isOutput=True)

    with tile.TileContext(nc) as tc:
        with (
            tc.tile_pool(name="wpool", bufs=1) as wpool,
            tc.tile_pool(name="xpool", bufs=3) as xpool,
            tc.tile_pool(name="rpool", bufs=6) as rpool,
            tc.tile_pool(name="opool", bufs=2) as opool,
            tc.tile_pool(name="psz", bufs=2, space=bass.MemorySpace.PSUM) as psz,
            tc.tile_pool(name="pso", bufs=4, space=bass.MemorySpace.PSUM) as pso,
        ):
            # resident weights
            wq_t = wpool.tile([128, NQ, FOUT], BF16, tag="wq")
            nc.sync.dma_start(wq_t[:], wq_d[:].transpose([1, 0, 2]))
            ds_t = wpool.tile([128, 2 * NQ, FOUT], BF16, tag="ds")
            nc.sync.dma_start(ds_t[:], ds_d[:].transpose([1, 0, 2]))
            bfp_t = wpool.tile([128, 2 * NQ], F32, tag="bfp")
            nc.sync.dma_start(bfp_t[:], bfp_d[:].transpose([1, 0, 2]).squeeze(2))
            bc2_t = wpool.tile([128, 1], F32, tag="bc2")
            nc.sync.dma_start(bc2_t[:], bc2_d[:])

            # align DMA queue round-robin phase to a multiple of 8 so the
            # recurring xs DMAs land on stable queues (no cross-queue WAW).
            pads_t = wpool.tile([1, 32], F32, tag="pads")
            for pi_ in range(4):
                nc.sync.dma_start(pads_t[0:1, pi_: pi_ + 1], bc2_d[0:1, 0:1])
            # warmups: each engine observes the weight-DMA semaphores once via
            # a tiny dummy op so real instructions never carry those waits.
            dummy = psz.tile([128, 2, 4, XC], F32, tag="z", name="zdummy")
            nc.tensor.matmul(
                dummy[0:64, 0, 0, 0:64], wq_t[0:9, 0, :], wq_t[0:9, 0, :],
                start=True, stop=True,
            )
            nc.tensor.matmul(
                dummy[0:64, 1, 0, 0:64], ds_t[:, 0, :], ds_t[:, 0, :],
                start=True, stop=True,
            )
            scr_a = wpool.tile([128, 1], F32, tag="scr_a")
            nc.scalar.activation(
                scr_a[:], bfp_t[:, 0:1],
                mybir.ActivationFunctionType.Relu, bias=bfp_t[:, 0:1],
            )
            scr_d1 = wpool.tile([128, 1], F32, tag="scr_d1")
            nc.vector.tensor_scalar(
                scr_d1[:], bc2_t[:], bc2_t[:, 0:1], None, mybir.AluOpType.add
            )
            scr_d2 = wpool.tile([128, 1], F32, tag="scr_d2")
            nc.vector.tensor_scalar(
                scr_d2[:], bfp_t[:, 0:1], bfp_t[:, 0:1], None, mybir.AluOpType.add
            )

            for gi, r0 in enumerate(CGRPS):
                # psum accumulators: po[0] holds chunks 0,1; po[1] chunks 2,3
                po = [pso.tile([128, 4, XC], F32, tag="po", name=f"po{gi}_{_pi}")
                      for _pi in range(2)]
                for q in range(NQ):
                    xs_t = xpool.tile([128, 16, XC], BF16, tag="xs")
                    for k in range(4):
                        # one contiguous DMA per channel (host im2col layout)
                        src = bass.AP(
                            xs_d,
                            (4 * q + k) * 9 * XR * XC + r0 * XC,
                            [[XR * XC, 9], [XC, 16], [1, XC]],
                        )
                        nc.sync.dma_start(xs_t[32 * k: 32 * k + 9, :, :], src)
                    for half in range(2):          # chunk pair (0,1) or (2,3)
                        for j in range(2):         # channel pair within quad
                            # conv: channel k=2j -> partitions 0:64 colgrp 0,
                            #       k=2j+1 -> partitions 64:128 colgrp 64
                            z_t = psz.tile([128, 2, 4, XC], F32, tag="z")
                            for ci in range(2):
                                c = 2 * half + ci
                                for kk_ in range(2):
                                    rg = 32 * (2 * j + kk_)
                                    nc.tensor.matmul(
                                        z_t[64 * kk_: 64 * kk_ + 64, ci, :, :],
                                        wq_t[rg: rg + 9, q, :],
                                        xs_t[rg: rg + 9, 4 * c: 4 * c + 4, :],
                                        start=True,
                                        stop=True,
                                        tile_position=(rg, 64 * kk_),
                                    )
                            # relu + per-partition bias -> bf16 SBUF
                            r_t = rpool.tile([128, 2, 4, XC], BF16, tag="r")
                            if j == 0:
                                nc.scalar.activation(
                                    r_t[:], z_t[:],
                                    mybir.ActivationFunctionType.Relu,
                                    bias=bfp_t[:, 2 * q + j: 2 * q + j + 1],
                                )
                            else:
                                nc.vector.tensor_scalar(
                                    r_t[:], z_t[:],
                                    bfp_t[:, 2 * q + j: 2 * q + j + 1], 0.0,
                                    mybir.AluOpType.add, mybir.AluOpType.max,
                                )
                            # combine: accumulate Wc-diag matmuls into po
                            for ci in range(2):
                                nc.tensor.matmul(
                                    po[half][64 * ci: 64 * ci + 64, :, :],
                                    ds_t[:, 2 * q + j, :],
                                    r_t[:, ci, :, :],
                                    start=(q == 0 and j == 0),
                                    stop=(q == NQ - 1 and j == 1),
                                    tile_position=(0, 64 * ci),
                                    skip_group_check=True,
                                )
                # evacuate: add bc, f32 out. Separate lo/hi tiles so each
                # absorber write soaks up exactly one out-DMA WAR semaphore;
                # the real evac ops then carry only the PE wait.
                obL = opool.tile([64, 2, 4, XC], F32, tag="obL")
                obH = opool.tile([128, 2, 4, XC], F32, tag="obH")
                nc.vector.tensor_scalar(
                    obL[0:1, 0, 0, 0:1], bc2_t[0:1, :], 0.0, None,
                    mybir.AluOpType.add,
                )
                nc.vector.tensor_scalar(
                    obH[64:65, 0, 0, 0:1], bc2_t[64:65, :], 0.0, None,
                    mybir.AluOpType.add,
                )
                for pi in range(2):
                    nc.vector.tensor_scalar(
                        obL[:, pi, :, :], po[pi][0:64, :, :], bc2_t[0:64, 0:1],
                        None, mybir.AluOpType.add,
                    )
                    nc.vector.tensor_scalar(
                        obH[64:128, pi, :, :], po[pi][64:128, :, :],
                        bc2_t[64:128, 0:1], None, mybir.AluOpType.add,
                    )
                # lo half: chunks 0,2 -> rows r0+{0..3, 8..11}
                dst_lo = bass.AP(
                    out_d, r0 * XC,
                    [[RPC * XC, FOUT], [8 * XC, 2], [XC, 4], [1, XC]],
                )
                nc.sync.dma_start(dst_lo, obL[:, :, :, :])
                if gi < 3:
                    dst_hi = bass.AP(
                        out_d, (r0 + 4) * XC,
                        [[RPC * XC, FOUT], [8 * XC, 2], [XC, 4], [1, XC]],
                    )
                    nc.sync.dma_start(dst_hi, obH[64:128, :, :, :])
                else:
                    dst_h1 = bass.AP(
                        out_d, (r0 + 4) * XC,
                        [[RPC * XC, FOUT], [XC, 4], [1, XC]],
                    )
                    nc.sync.dma_start(dst_h1, obH[64:128, 0, :, :])
                    dst_h3 = bass.AP(
                        out_d, (r0 + 12) * XC,
                        [[RPC * XC, FOUT], [XC, 3], [1, XC]],
                    )
                    nc.sync.dma_start(dst_h3, obH[64:128, 1, 0:3, :])
                n_out_dmas = 2 if gi < 3 else 3
                for pi_ in range(8 - n_out_dmas):
                    idx = 4 + gi * 6 + pi_
                    nc.sync.dma_start(pads_t[0:1, idx: idx + 1], bc2_d[0:1, 0:1])
    if hoist:
        _hoist_extra_waits(nc)
    return nc


_NC = None


def _get_nc():
    global _NC
    if _NC is None:
        _NC = _build()
    return _NC


def _pack_weights(Wf, bf, Wc, bc):
    Wf_t = Wf.transpose(1, 2, 3, 0).reshape(FIN, 9, FOUT)  # [i, t, o]
    wq = np.zeros((NQ, 128, FOUT), np.float32)
    for k in range(4):
        wq[:, 32 * k: 32 * k + 9, :] = Wf_t[np.arange(NQ) * 4 + k]
    ds = np.zeros((2 * NQ, 128, FOUT), np.float32)
    eye = np.eye(FOUT, dtype=np.float32)
    for p in range(2 * NQ):  # p = 2q+j ; channels (4q+2j, 4q+2j+1)
        q, j = p // 2, p % 2
        ds[p, 0:64, :] = eye * Wc[:, 4 * q + 2 * j][None, :]
        ds[p, 64:128, :] = eye * Wc[:, 4 * q + 2 * j + 1][None, :]
    bfp = np.zeros((2 * NQ, 128, 1), np.float32)
    for p in range(2 * NQ):
        q, j = p // 2, p % 2
        bfp[p, 0:64, 0] = bf[:, 4 * q + 2 * j]
        bfp[p, 64:128, 0] = bf[:, 4 * q + 2 * j + 1]
    bc2 = np.tile(bc.reshape(64, 1), (2, 1)).astype(np.float32)
    return {
        "wq": wq.astype(npbf16),
        "ds": ds.astype(npbf16),
        "bfp": bfp,
        "bc2": bc2,
    }


def _im2col(x, b, h):
    """[FIN, 9, XR, XC] bf16: xs[i, 3*dy+dx, r, c] = x[b, i, 63h+r+dy, c+dx]
    (zero-padded outside the valid range)."""
    xpad = np.zeros((FIN, XR + KK - 1, XC + KK - 1), np.float32)
    row_hi = min(H, RPC * h + XR + KK - 1)
    nrows = row_hi - RPC * h
    xpad[:, 0:nrows, 0:W] = x[b, :, RPC * h: row_hi, :]
    sw = np.lib.stride_tricks.sliding_window_view(xpad, (KK, KK), axis=(1, 2))
    return np.ascontiguousarray(
        sw.transpose(0, 3, 4, 1, 2).reshape(FIN, 9, XR, XC)
    ).astype(npbf16)


def _run(x, Wf, bf, Wc, bc, **spmd_kwargs):
    shared = _pack_weights(Wf, bf, Wc, bc)
    in_maps = []
    for core in range(8):
        b, h = core // 2, core % 2
        m = dict(shared)
        m["xs"] = _im2col(x, b, h)
        in_maps.append(m)
    res = run_bass_kernel_spmd(_get_nc(), in_maps, list(range(8)), **spmd_kwargs)
    out = np.empty((B, FOUT, HO, WO), np.float32)
    for core in range(8):
        b, h = core // 2, core % 2
        out[b, :, RPC * h: RPC * h + RPC, :] = np.asarray(
            res.results[core]["out"], np.float32
        )[:, :, 0:WO]
    return out, res


def kernel(x, Wf, bf, Wc, bc):
    x = np.asarray(x, np.float32)
    out, _ = _run(
        x,
        np.asarray(Wf, np.float32),
        np.asarray(bf, np.float32),
        np.asarray(Wc, np.float32),
        np.asarray(bc, np.float32),
    )
    return out



# revision 7
# speedup vs baseline: 208.8636x; 208.8636x over previous
"""Trainium2 Bass kernel for depthwise-multiplier conv + ReLU + per-out-channel
1x1 combine (nn_Comb_70016556859799).

Math (reference):
  out[b,o,p,q] = bc[o] + sum_i Wc[o,i] * relu( sum_{dy,dx} Wf[o,i,dy,dx]*x[b,i,p+dy,q+dx] + bf[o,i] )

Sharding: 8 cores = (batch b in 0..3) x (H half in 0..1). Each core computes
out[b, :, 63*h : 63*h+63, :] from x[b, :, 63*h : 63*h+66, :].

Per-core dataflow (K=126 im2col packing):
  - input: padded x slice [70ch, 67*128] bf16 in HBM. Five resident SBUF
    tiles [126, 8192]: partition (14*t + cl)*9 + tap holds channel 14t+cl
    shifted by tap offset (dy*128+dx) -- the 9x im2col replication is done
    by the load DMA itself via overlapping 16KB descriptors.
  - conv: for each channel pair p (2 channels x 64 outs = 128 planes) and
    512-pixel chunk: one matmul z[128,512] = w2[:,p,:]^T(126x128) @ xs. The
    lhsT is zero outside the pair's 18 rows, so each column cycle yields 128
    conv outputs (the PE column-stream optimum).
  - relu+bias -> bf16 SBUF, split across Act/DVE/Pool engines (13/11/8).
  - combine: po[64*ch..][512] += ds[:,p,:]^T(128x64 stacked-diag Wc) @ r,
    PSUM-accumulated over the 32 pairs; two 4-row chunks share one PSUM bank
    (partitions 0:64 / 64:128).
  - evac: += bc on ScalarE, one 2-chunk output DMA per chunk-group.
PE emission is software-pipelined (combine lags conv by 2 steps); PSUM uses
all 8 banks (3x2 z double-buffers + 2x1 po).
"""
import numpy as np
import ml_dtypes

import concourse.bass as bass
import concourse.mybir as mybir
from concourse import tile
from concourse.bass_utils import run_bass_kernel_spmd

BF16 = mybir.dt.bfloat16
F32 = mybir.dt.float32
npbf16 = ml_dtypes.bfloat16

B, FIN, FOUT, KK, H, W = 4, 64, 64, 3, 128, 128
HO, WO = H - KK + 1, W - KK + 1          # 126, 126
RPC = HO // 2                             # 63 output rows per core
XC = 128                                  # padded row width
NPAIR = FIN // 2                          # 32 channel pairs
NTILE = 5                                 # xs tiles of 14 channels each
CPT = 14                                  # channels per tile
PPT = 7                                   # pairs per tile
NCG = 8                                   # chunk groups (2 chunks each)
CHUNK = 512                               # pixels per chunk (4 rows x 128)
XROW = 67                                 # padded input rows per core
PIPE = 2                                  # combine lags conv by PIPE steps

# relu engine schedule: weighted by engine speed (Act .833, DVE 1.042,
# Pool 1.389 ns/elem) -> 13/11/8 split, error-diffusion interleaved.
def _relu_order():
    counts = {"a": 13, "d": 11, "p": 8}
    acc = {e: 0.0 for e in counts}
    order = []
    for _ in range(NPAIR):
        for e in counts:
            acc[e] += counts[e] / float(NPAIR)
        pick = max(acc, key=lambda e: (acc[e], e))
        acc[pick] -= 1.0
        order.append(pick)
    return order

RELU_ORDER = _relu_order()


def _hoist_extra_waits(nc):
    """Walrus supports only one sync-wait command per instruction. Tile can
    emit several (multiple producer procs). Hoist all but the last wait onto
    fresh same-engine NoOp instructions placed immediately before -- the waits
    still execute on the same engine sequencer in the same order, so the
    synchronization semantics are unchanged."""
    import copy
    n_hoist = 0
    for blk in nc.m.functions[0].blocks:
        newinsts = []
        for inst in blk.instructions:
            si = getattr(inst, "sync_info", None)
            ow = list(si.on_wait) if si is not None and si.on_wait else []
            if len(ow) > 1:
                for wi, w in enumerate(ow[:-1]):
                    nop = mybir.InstNoOp(
                        name=f"{inst.name}_hw{wi}",
                        text_hint="hoisted_wait",
                        bass_nofuse=True,
                    )
                    nop.engine = inst.engine
                    nsi = copy.deepcopy(si)
                    nsi.on_wait = [w]
                    if getattr(nsi, "on_update", None):
                        nsi.on_update = []
                    nop.sync_info = nsi
                    newinsts.append(nop)
                    n_hoist += 1
                si.on_wait = [ow[-1]]
            newinsts.append(inst)
        blk.instructions = newinsts
    return n_hoist


def _build(hoist=True):
    nc = bass.Bass()
    xs_d = nc.declare_dram_parameter("xs", [CPT * NTILE * KK, XROW * XC], BF16,
                                     isOutput=False)
    w2_d = nc.declare_dram_parameter("w2", [126, NPAIR, 128], BF16,
                                     isOutput=False)
    ds_d = nc.declare_dram_parameter("ds", [128, NPAIR, FOUT], BF16,
                                     isOutput=False)
    bfp_d = nc.declare_dram_parameter("bfp", [128, NPAIR], F32, isOutput=False)
    bc2_d = nc.declare_dram_parameter("bc2", [128, 1], F32, isOutput=False)
    out_d = nc.declare_dram_parameter("out", [FOUT, 64, XC], F32, isOutput=True)

    AF = mybir.ActivationFunctionType
    ALU = mybir.AluOpType

    with tile.TileContext(nc) as tc:
        with (
            tc.tile_pool(name="wpool", bufs=1) as wpool,
            tc.tile_pool(name="xpool", bufs=1) as xpool,
            tc.tile_pool(name="rpool", bufs=3) as rpool,
            tc.tile_pool(name="opool", bufs=2) as opool,
            tc.tile_pool(name="psz", bufs=3, space=bass.MemorySpace.PSUM) as psz,
            tc.tile_pool(name="pso", bufs=2, space=bass.MemorySpace.PSUM) as pso,
        ):
            # resident weights
            w2_t = wpool.tile([126, NPAIR, 128], BF16, tag="w2")
            nc.sync.dma_start(w2_t[:], w2_d[:])
            ds_t = wpool.tile([128, NPAIR, FOUT], BF16, tag="ds")
            nc.sync.dma_start(ds_t[:], ds_d[:])
            bfp_t = wpool.tile([128, NPAIR], F32, tag="bfp")
            nc.sync.dma_start(bfp_t[:], bfp_d[:])
            bc2_t = wpool.tile([128, 1], F32, tag="bc2")
            nc.sync.dma_start(bc2_t[:], bc2_d[:])

            # resident im2col tiles: one DMA each; src descriptors overlap to
            # replicate each channel 9x with tap shifts.
            xs_t = []
            for t in range(NTILE):
                xt = xpool.tile([126, NCG, 2, CHUNK], BF16, tag=f"xs{t}")
                src = bass.AP(
                    xs_d,
                    CPT * KK * t * XROW * XC,
                    [[XROW * XC, CPT * KK], [1, KK], [1, NCG * 2 * CHUNK]],
                )
                nc.sync.dma_start(xt[:], src)
                xs_t.append(xt)

            # warmups: each engine observes the weight-DMA semaphores once via
            # a tiny dummy op so real instructions never carry those waits.
            dummy = psz.tile([128, 2, CHUNK], F32, tag="z", name="zdummy")
            nc.tensor.matmul(
                dummy[0:64, 0, 0:64], w2_t[0:18, 0, 0:64], w2_t[0:18, 1, 0:64],
                start=True, stop=True,
            )
            nc.tensor.matmul(
                dummy[0:64, 1, 0:64], ds_t[:, 0, :], ds_t[:, 1, 0:64],
                start=True, stop=True,
            )
            scr_a = wpool.tile([128, 1], F32, tag="scr_a")
            nc.scalar.activation(
                scr_a[:], bc2_t[:], AF.Relu, bias=bfp_t[:, 0:1],
            )
            scr_d = wpool.tile([128, 1], F32, tag="scr_d")
            nc.vector.tensor_scalar(
                scr_d[:], bc2_t[:], bfp_t[:, 0:1], None, ALU.add
            )
            scr_p = wpool.tile([128, 1], F32, tag="scr_p")
            nc.gpsimd.tensor_scalar(
                scr_p[:], bc2_t[:], bfp_t[:, 0:1], None, ALU.add
            )

            # main pipeline over flat steps s = (cg, p)
            pending = []          # (cg, p, r_tile)
            po = [None, None]     # rotating po tiles by cg parity

            def drain_one():
                cg, p, r_t = pending.pop(0)
                if p == 0:
                    po[cg % 2] = pso.tile([128, CHUNK], F32, tag="po",
                                          name=f"po{cg}")
                pot = po[cg % 2]
                for ch in range(2):
                    nc.tensor.matmul(
                        pot[64 * ch: 64 * ch + 64, :],
                        ds_t[:, p, :],
                        r_t[:, ch, :],
                        start=(p == 0),
                        stop=(p == NPAIR - 1),
                        tile_position=(0, 64 * ch),
                        skip_group_check=True,
                    )
                if p == NPAIR - 1:
                    ob = opool.tile([128, CHUNK], F32, tag="ob", name=f"ob{cg}")
                    nc.scalar.activation(
                        ob[:], pot[:], AF.Identity, bias=bc2_t[:, 0:1],
                    )
                    dst = bass.AP(
                        out_d, 8 * cg * XC,
                        [[4 * XC, 2], [64 * XC, FOUT], [1, CHUNK]],
                    )
                    nc.sync.dma_start(dst, ob[:])

            for cg in range(NCG):
                for p in range(NPAIR):
                    t = p // PPT
                    z_t = psz.tile([128, 2, CHUNK], F32, tag="z")
                    for ch in range(2):
                        nc.tensor.matmul(
                            z_t[:, ch, :],
                            w2_t[:, p, :],
                            xs_t[t][:, cg, ch, :],
                            start=True, stop=True,
                        )
                    r_t = rpool.tile([128, 2, CHUNK], BF16, tag="r")
                    eng = RELU_ORDER[p]
                    if eng == "a":
                        nc.scalar.activation(
                            r_t[:], z_t[:], AF.Relu, bias=bfp_t[:, p: p + 1],
                        )
                    elif eng == "d":
                        nc.vector.tensor_scalar(
                            r_t[:], z_t[:], bfp_t[:, p: p + 1], 0.0,
                            ALU.add, ALU.max,
                        )
                    else:
                        nc.gpsimd.tensor_scalar(
                            r_t[:], z_t[:], bfp_t[:, p: p + 1], 0.0,
                            ALU.add, ALU.max,
                        )
                    pending.append((cg, p, r_t))
                    if len(pending) > PIPE:
                        drain_one()
            while pending:
                drain_one()
    if hoist:
        _hoist_extra_waits(nc)
    return nc


_NC = None


def _get_nc():
    global _NC
    if _NC is None:
        _NC = _build()
    return _NC


def _pack_weights(Wf, bf, Wc, bc):
    # conv lhsT: [126, 32 pairs, 128]; pair p covers channels (2p, 2p+1),
    # nonzero rows 18*(p%7) .. +18 (matching its xs tile partitions),
    # col j = 64*jc + o -> Wf[o, 2p+jc, dy, dx] at row offset 9*jc + 3*dy+dx.
    w2 = np.zeros((126, NPAIR, 128), np.float32)
    Wf_flat = Wf.reshape(FOUT, FIN, 9)  # [o, c, tap]
    for p in range(NPAIR):
        i = p % PPT
        for jc in range(2):
            c = 2 * p + jc
            # rows 18i+9jc+tap, cols 64jc+o
            w2[18 * i + 9 * jc: 18 * i + 9 * jc + 9, p, 64 * jc: 64 * jc + 64] = (
                Wf_flat[:, c, :].T
            )
    # combine lhsT: stacked-diagonal Wc pairs [128, 32, 64]
    ds = np.zeros((128, NPAIR, FOUT), np.float32)
    eye = np.eye(FOUT, dtype=np.float32)
    for p in range(NPAIR):
        ds[0:64, p, :] = eye * Wc[:, 2 * p][None, :]
        ds[64:128, p, :] = eye * Wc[:, 2 * p + 1][None, :]
    # conv bias per plane: [128, 32]
    bfp = np.zeros((128, NPAIR), np.float32)
    for p in range(NPAIR):
        bfp[0:64, p] = bf[:, 2 * p]
        bfp[64:128, p] = bf[:, 2 * p + 1]
    bc2 = np.tile(bc.reshape(FOUT, 1), (2, 1)).astype(np.float32)
    return {
        "w2": w2.astype(npbf16),
        "ds": ds.astype(npbf16),
        "bfp": bfp,
        "bc2": bc2,
    }


def _im2col(x, b, h):
    """Padded per-core x slice [210, 67*128] bf16: row 3c+dy holds channel c
    (of 70 = 64 real + 6 zero-pad) shifted by dy rows. The dx shift is done
    by the load DMA's overlapping descriptors."""
    xp = np.zeros((CPT * NTILE, XROW * XC), np.float32)
    r0 = RPC * h
    nrows = min(H - r0, XROW)
    xp[0:FIN, 0: nrows * XC] = x[b, :, r0: r0 + nrows, :].reshape(FIN, -1)
    out = np.zeros((CPT * NTILE * KK, XROW * XC), np.float32)
    for dy in range(KK):
        ncols = XROW * XC - dy * XC
        out[dy::KK, 0:ncols] = xp[:, dy * XC:]
    return np.ascontiguousarray(out).astype(npbf16)


def _run(x, Wf, bf, Wc, bc, **spmd_kwargs):
    shared = _pack_weights(Wf, bf, Wc, bc)
    in_maps = []
    for core in range(8):
        b, h = core // 2, core % 2
        m = dict(shared)
        m["xs"] = _im2col(x, b, h)
        in_maps.append(m)
    res = run_bass_kernel_spmd(_get_nc(), in_maps, list(range(8)), **spmd_kwargs)
    out = np.empty((B, FOUT, HO, WO), np.float32)
    for core in range(8):
        b, h = core // 2, core % 2
        out[b, :, RPC * h: RPC * h + RPC, :] = np.asarray(
            res.results[core]["out"], np.float32
        )[:, 0:RPC, 0:WO]
    return out, res


def kernel(x, Wf, bf, Wc, bc):
    x = np.asarray(x, np.float32)
    out, _ = _run(
        x,
        np.asarray(Wf, np.float32),
        np.asarray(bf, np.float32),
        np.asarray(Wc, np.float32),
        np.asarray(bc, np.float32),
    )
    return out
